# revision 1
# baseline (speedup 1.0000x reference)
"""Causal self-attention Trainium2 kernel (B=2, T=2048, C=1024, H=16, D=64).

Sharding: 8 cores = data-parallel on B (2) x tensor-parallel on heads (16/4=4
heads per core). Column-parallel Wqkv, row-parallel Wproj; the row-parallel
partial outputs are summed on the host.

v3 design (bf16 datapath, fp32 PSUM accumulation, 256-token slices):
  - x, Wqkv, Wproj are cast to bf16 on the host. bf16 matmuls stream at 1
    column/cycle at any free width; SBUF/DMA traffic halves.
  - x is shipped host-transposed (feature-major [p, cs, t]) so x^T tiles
    arrive via plain DMA copies: no PE transposes and no xbar-transpose
    instructions (whose Tile-level serialization against other DMAs costs
    ~1.3us per kind-alternation).
  - Q^T, K^T come from feature-major qkv matmuls (lhsT = W chunk, rhs = x^T).
    V is computed directly in its t-major PV layout (lhsT = x^T tile,
    rhs = Wv).
  - Flash-style attention per head in S^T ([k, q]) orientation; exp on ACT
    (scale=1/8 folded in; logits ~ N(0,1) so no max subtraction); causal mask
    via affine_select on GpSimd; PV accumulation with lhsT = V_aug (65th row
    of ones accumulates the softmax denominator). The two heads of a
    128-partition pair use contraction rows 0-63/64-127, so their S^T matmuls
    land in different PE row-groups (tile_position) and overlap in the array.
  - 256-token slices keep every PSUM pool dual/quad-buffered in its own
    bank(s) (ps 4, py 2, qkv 1, proj 1 = 8 banks), so the S -> exp -> PV
    chain pipelines instead of ping-ponging PE against ACT. PV is deferred
    by one k-tile per head-pair, and projection / qkv chunks of the
    neighbouring slices are interleaved between attention tiles to fill the
    PE while ACT works through the exps.
  - Normalization: reciprocal of the denominator row on DVE, broadcast
    across the 64 d-partitions on GpSimd (partition_broadcast), multiplied
    on DVE; deferred into the next slice so PSUM stays free during attention.
  - Row-parallel projection lhsT = y^T; bf16 outputs are summed on the host.
"""

import numpy as np

import concourse.bacc as bacc
import concourse.mybir as mybir
import concourse.tile as tile
from concourse.bass_utils import run_bass_kernel_spmd

B, T, C, H, D = 2, 2048, 1024, 16, 64
NCORES = 8
HPC = H // (NCORES // B)  # 4 heads per core
DSH = HPC * D             # 256 head-dims per core
P = 128
TS = 256                  # q/t slice width
NTS = T // TS             # 8 slices
NT = T // P               # 16 k-tiles
CS = C // P               # 8 contraction subtiles
TPS = TS // P             # 2 t-tiles per slice

f32 = mybir.dt.float32
bf16 = mybir.dt.bfloat16
FP = mybir.ActivationFunctionType
NPBF16 = mybir.dt.np(bf16)


def build_program(reps=1, use_bias=False):
    nc = bacc.Bacc("TRN2", debug=False, num_devices=NCORES)
    # host-transposed x: x[p, cs, t] = X[t, cs*128+p] — plain DMA loads, no
    # xbar-transpose (whose Tile serialization vs DMACopies costs us ~1.3us
    # per kind-alternation)
    x_d = nc.dram_tensor("x", [P, CS, T], bf16, kind="ExternalInput").ap()
    # chunk-major host layout: wqkv[ch, p, cs, fo] = Wqkv[cs*128+p, ch*128+fo]
    wqkv_d = nc.dram_tensor("wqkv", [6, P, CS, P], bf16, kind="ExternalInput").ap()
    bqkv_d = nc.dram_tensor("bqkv", [3 * DSH], f32, kind="ExternalInput").ap()
    wproj_d = nc.dram_tensor("wproj", [DSH, C], bf16, kind="ExternalInput").ap()
    out_d = nc.dram_tensor("out", [T, C], bf16, kind="ExternalOutput").ap()

    with tile.TileContext(nc) as tc:
        for _ in range(reps):
            kernel_body(tc, x_d, wqkv_d, bqkv_d, wproj_d, out_d, use_bias)
    nc.compile()
    return nc


def kernel_body(tc, x_d, wqkv_d, bqkv_d, wproj_d, out_d, use_bias=False):
    nc = tc.nc
    from contextlib import ExitStack

    ctx = ExitStack()
    with ctx:
        consts = ctx.enter_context(tc.tile_pool(name="consts", bufs=1))
        bias_col = consts.tile([P, 4], f32)
        bias_v = consts.tile([P, DSH], f32)
        bias_v1 = consts.tile([1, DSH], f32)

        persist = ctx.enter_context(tc.tile_pool(name="persist", bufs=1))
        wq_sb = persist.tile([P, 6, CS, P], bf16)
        kT_sb = persist.tile([P, 2, T], bf16)
        vaug = persist.tile([P, NT, HPC, 65], bf16)
        nc.vector.memset(vaug[:, :, :, 64], 1.0)
        yT = persist.tile([P, 2, T], bf16)
        wp_sb = persist.tile([P, 2, C], bf16)
        wq_src = wqkv_d.rearrange("ch p cs f -> p ch cs f")

        with (
            tc.tile_pool(name="xsb", bufs=8) as xsb_pool,
            tc.tile_pool(name="qts", bufs=2) as qts_pool,
            tc.tile_pool(name="expS", bufs=6) as expS_pool,
            tc.tile_pool(name="bc", bufs=6) as bc_pool,
            tc.tile_pool(name="outsb", bufs=4) as outsb_pool,
            tc.tile_pool(name="pmm", bufs=2, space="PSUM") as pmm_pool,
            tc.tile_pool(name="ps", bufs=2, space="PSUM") as ps_pool,
            tc.tile_pool(name="py", bufs=2, space="PSUM") as py_pool,
        ):
            def emit_qk(ch, si, qTs, xTs):
                pq = pmm_pool.tile([P, TS], f32, name="pq", tag="pmm")
                for cs in range(CS):
                    nc.tensor.matmul(
                        pq,
                        lhsT=wq_sb[:, ch, cs, :],
                        rhs=xTs[:, cs, :],
                        start=(cs == 0),
                        stop=(cs == CS - 1),
                    )
                if ch < 2:
                    dst = qTs[:, ch, :]
                else:
                    dst = kT_sb[:, ch - 2, si * TS : (si + 1) * TS]
                if use_bias:
                    nc.vector.tensor_scalar_add(dst, pq, bias_col[:, ch : ch + 1])
                else:
                    nc.vector.tensor_copy(dst, pq)

            def emit_v(si, a, xTs):
                kt = TPS * si + a
                pv = pmm_pool.tile([P, DSH], f32, name="pv", tag="pmm")
                for cs in range(CS):
                    nc.tensor.matmul(
                        pv,
                        lhsT=xTs[:, cs, a * P : (a + 1) * P],
                        rhs=wq_sb[:, 4:6, cs, :],
                        start=(cs == 0),
                        stop=(cs == CS - 1),
                    )
                dst = vaug[:, kt, :, 0:64]
                src = pv.rearrange("p (h d) -> p h d", h=HPC)
                if use_bias:
                    nc.vector.tensor_add(
                        dst, src, bias_v.rearrange("p (h d) -> p h d", h=HPC)
                    )
                else:
                    nc.vector.tensor_copy(dst, src)

            # Deferred PV per head-pair: emit S+exp for a tile (or hist pair),
            # then flush the pending PVs of the previous tile, keeping PE
            # ahead of ACT. Entries: (si, py_t, [(kt, qoff, [rhs_hh0, rhs_hh1])]).
            pend_pv = [None, None]

            def emit_pv(hp):
                if pend_pv[hp] is None:
                    return
                si, py_t, entries = pend_pv[hp]
                pend_pv[hp] = None
                n_k = TPS * (si + 1)
                for kt, qoff, rhss in entries:
                    for hh in range(2):
                        # the two heads share one PSUM bank: only the first
                        # matmul of the group clears it, only the last stops it
                        nc.tensor.matmul(
                            py_t[:65, hh, qoff:TS],
                            lhsT=vaug[:, kt, 2 * hp + hh, :],
                            rhs=rhss[hh],
                            start=(kt == 0 and hh == 0),
                            stop=(kt == n_k - 1 and hh == 1),
                        )

            def emit_s_pair(si, hp, kt0, qTs, py01):
                # two full-width history k-tiles fused into one exp instruction
                ps_t = ps_pool.tile([P, 2, 2, TS], f32, name="ps_t", tag="ps")
                ex_t = expS_pool.tile([P, 2, 2, TS], bf16, name="ex_t")
                for par in range(2):
                    for hh in range(2):
                        hb = hh * 64
                        nc.tensor.matmul(
                            ps_t[:, hh, par, :],
                            lhsT=kT_sb[hb : hb + 64, hp, (kt0 + par) * P : (kt0 + par + 1) * P],
                            rhs=qTs[hb : hb + 64, hp, :],
                            start=True,
                            stop=True,
                            tile_position=(hb, 0),
                        )
                nc.scalar.activation(ex_t, ps_t, FP.Exp, scale=0.125)
                emit_pv(hp)
                pend_pv[hp] = (
                    si,
                    py01,
                    [
                        (kt0, 0, [ex_t[:, 0, 0, :], ex_t[:, 1, 0, :]]),
                        (kt0 + 1, 0, [ex_t[:, 0, 1, :], ex_t[:, 1, 1, :]]),
                    ],
                )

            def emit_s(si, hp, kt, qTs, py01):
                qoff = max(0, kt * P - si * TS)
                # pad each head's half to its own 2KB PSUM bank so the two
                # row-group-tiled S matmuls can drain concurrently
                ps_t = ps_pool.tile(
                    [P, 2, TS], f32, name="ps_t", tag="ps", padded_shape=[P, 2, 512]
                )
                ex_t = expS_pool.tile([P, 2, TS], bf16, name="ex_t")
                for hh in range(2):
                    hb = hh * 64
                    nc.tensor.matmul(
                        ps_t[:, hh, qoff:TS],
                        lhsT=kT_sb[hb : hb + 64, hp, kt * P : (kt + 1) * P],
                        rhs=qTs[hb : hb + 64, hp, qoff:TS],
                        start=True,
                        stop=True,
                        tile_position=(hb, 0),
                    )
                nc.scalar.activation(
                    ex_t[:, :, qoff:TS], ps_t[:, :, qoff:TS], FP.Exp, scale=0.125
                )
                if kt >= TPS * si:  # zero k > q in the diagonal 128-col block
                    for hh in range(2):
                        nc.gpsimd.affine_select(
                            out=ex_t[:, hh, qoff : qoff + P],
                            in_=ex_t[:, hh, qoff : qoff + P],
                            compare_op=mybir.AluOpType.is_ge,
                            fill=0.0,
                            base=0,
                            channel_multiplier=-1,
                            pattern=[[1, P]],
                        )
                emit_pv(hp)
                pend_pv[hp] = (
                    si,
                    py01,
                    [(kt, qoff, [ex_t[:, 0, qoff:TS], ex_t[:, 1, qoff:TS]])],
                )

            def emit_norm(p):
                f_si, f_qsl, f_py0, f_py1 = p
                for hp, py_t in ((0, f_py0), (1, f_py1)):
                    for hh in range(2):
                        hb = hh * 64
                        rc_t = bc_pool.tile([1, TS], f32, name="rc_t", tag="rc")
                        nc.vector.reciprocal(rc_t, py_t[64:65, hh, :])
                        bc_t = bc_pool.tile([64, TS], f32, name="bc_t", tag="bc")
                        nc.gpsimd.partition_broadcast(bc_t, rc_t, channels=64)
                        nc.vector.tensor_mul(
                            yT[hb : hb + 64, hp, f_qsl], py_t[0:64, hh, :], bc_t
                        )

            def emit_proj(f_si, qq, ob_t):
                qt = f_si * TPS + qq
                for cc in range(4):
                    po_t = pmm_pool.tile([P, TS], f32, name="po_t", tag="pmm")
                    for chp in range(2):
                        nc.tensor.matmul(
                            po_t,
                            lhsT=yT[:, chp, qt * P : (qt + 1) * P],
                            rhs=wp_sb[:, chp, cc * TS : (cc + 1) * TS],
                            start=(chp == 0),
                            stop=(chp == 1),
                        )
                    nc.vector.tensor_copy(ob_t[:, cc * TS : (cc + 1) * TS], po_t)
                nc.sync.dma_start(out_d[qt * P : (qt + 1) * P, :], ob_t)

            def py_pair():
                return py_pool.tile([P, 2, TS], f32, name="py", tag="py")

            pending = None
            proj_fill = []  # deferred proj units, carried across slices
            xtiles = {}

            def x_load(sj):
                t_ = xsb_pool.tile([P, CS, TS], bf16, name="x_sb")
                nc.sync.dma_start(t_, x_d[:, :, sj * TS : (sj + 1) * TS])
                xtiles[sj] = t_

            nc.sync.dma_start(wq_sb[:, 0], wq_src[:, 0])
            x_load(0)
            for ch in range(1, 4):
                nc.sync.dma_start(wq_sb[:, ch], wq_src[:, ch])
            nc.sync.dma_start(wq_sb[:, 4:6], wq_src[:, 4:6])
            x_load(1)
            nc.sync.dma_start(wp_sb, wproj_d.rearrange("(ch p) f -> p ch f", p=P))
            for sj in range(2, NTS):
                x_load(sj)
            for si in range(NTS):
                xTs = xtiles.pop(si)
                qTs = qts_pool.tile([P, 2, TS], bf16, name="qTs")
                if si == 0:
                    if use_bias:
                        nc.sync.dma_start(
                            bias_col,
                            bqkv_d[0 : 4 * P].rearrange("(ch p) -> p ch", p=P),
                        )
                        nc.sync.dma_start(
                            bias_v1, bqkv_d[2 * DSH : 3 * DSH].rearrange("f -> 1 f")
                        )
                        nc.gpsimd.partition_broadcast(bias_v, bias_v1, channels=P)
                emit_qk(0, si, qTs, xTs)
                emit_qk(1, si, qTs, xTs)
                py01s = [py_pair(), py_pair()]
                if pending is not None:
                    emit_norm(pending)

                # filler units: previous slices' projections + this slice's K/V
                if pending is not None:
                    f_si = pending[0]
                    for qq in range(TPS):
                        proj_fill.append((f_si, qq))
                pending = None
                kv_fill = [("qk", 2), ("qk", 3), ("v", 0), ("v", 1)]

                # hp-blocked order: with 2 ps slots this gives depth-2
                # run-ahead within each head-pair's S->exp chain.
                # History tiles come in fused kt-pairs (2si is always even).
                hist_units = [
                    (hp, kt0) for hp in range(2) for kt0 in range(0, TPS * si, 2)
                ]
                diag_units = [
                    (hp, kt)
                    for hp in range(2)
                    for kt in range(TPS * si, TPS * (si + 1))
                ]

                def pop_filler(allow_kv=True, allow_proj=True):
                    if allow_kv and kv_fill:
                        f = kv_fill.pop(0)
                        if f[0] == "qk":
                            emit_qk(f[1], si, qTs, xTs)
                        else:
                            emit_v(si, f[1], xTs)
                        return True
                    if allow_proj and proj_fill:
                        f_si, qq = proj_fill.pop(0)
                        ob_t = outsb_pool.tile([P, C], bf16, name="ob_t")
                        emit_proj(f_si, qq, ob_t)
                        return True
                    return False

                for i, (hp, kt0) in enumerate(hist_units):
                    emit_s_pair(si, hp, kt0, qTs, py01s[hp])
                    pop_filler()
                # K/V for this slice must be complete before the diagonal
                while pop_filler(allow_proj=False):
                    pass
                for i, (hp, kt) in enumerate(diag_units):
                    emit_s(si, hp, kt, qTs, py01s[hp])
                    if i % 2 == 1:
                        pop_filler(allow_kv=False)
                emit_pv(0)
                emit_pv(1)
                pending = (si, slice(si * TS, (si + 1) * TS), py01s[0], py01s[1])

            emit_norm(pending)
            proj_fill.append((pending[0], 0))
            proj_fill.append((pending[0], 1))
            for f_si, qq in proj_fill:
                ob_t = outsb_pool.tile([P, C], bf16, name="ob_t")
                emit_proj(f_si, qq, ob_t)


_NC_CACHE = {}


def get_program(use_bias=False):
    key = ("nc", use_bias)
    if key not in _NC_CACHE:
        _NC_CACHE[key] = build_program(use_bias=use_bias)
    return _NC_CACHE[key]


def shard_inputs(x, w_qkv, b_qkv, w_proj):
    """Per-core input dicts: core c -> batch c//4, head-group c%4."""
    x = np.asarray(x, dtype=np.float32).astype(NPBF16)
    w_qkv = np.asarray(w_qkv, dtype=np.float32).astype(NPBF16)
    b_qkv = np.asarray(b_qkv, dtype=np.float32)
    w_proj = np.asarray(w_proj, dtype=np.float32).astype(NPBF16)
    in_maps = []
    for c in range(NCORES):
        b, g = divmod(c, NCORES // B)
        cols = []
        for r_ in range(3):  # q, k, v regions
            lo = r_ * C + g * DSH
            cols.append(np.arange(lo, lo + DSH))
        cols = np.concatenate(cols)
        wq = w_qkv[:, cols]  # [C, 3*DSH]
        # chunk-major device layout: [ch, p, cs, fo]
        wq_dev = np.ascontiguousarray(
            wq.reshape(CS, P, 6, P).transpose(2, 1, 0, 3)
        )
        in_maps.append(
            {
                "x": np.ascontiguousarray(
                    x[b].reshape(T, CS, P).transpose(2, 1, 0)
                ),
                "wqkv": wq_dev,
                "bqkv": np.ascontiguousarray(b_qkv[cols]),
                "wproj": np.ascontiguousarray(w_proj[g * DSH : (g + 1) * DSH, :]),
            }
        )
    return in_maps


def kernel(x, w_qkv, b_qkv, w_proj, b_proj, _trace=False):
    use_bias = bool(np.any(np.asarray(b_qkv)))
    nc = get_program(use_bias)
    in_maps = shard_inputs(x, w_qkv, b_qkv, w_proj)
    res = run_bass_kernel_spmd(nc, in_maps, core_ids=list(range(NCORES)), trace=_trace)
    out = np.zeros((B, T, C), dtype=np.float32)
    for c in range(NCORES):
        out[c // (NCORES // B)] += res.results[c]["out"].astype(np.float32)
    out += np.asarray(b_proj, dtype=np.float32)[None, None, :]
    if _trace:
        kernel._last_results = res
    return out

